# revision 12
# baseline (speedup 1.0000x reference)
"""BigResNet Trainium2 kernel (bf16, drain-optimized).

Computation (see reference): x:[65536,100]; 100 blocks of
(10x Linear(100,100)+ReLU) with a residual add per block; final Linear(100,10).

Strategy:
- Data-parallel over the batch: 8 cores x 8192 rows each.
- On TRN2 this problem is bound by the PSUM->SBUF ReLU drain: only ACT and
  DVE can read PSUM (Pool/GpSimd and DMA are locked out by the ISA), each
  moving ~1 element/lane/cycle. The kernel keeps both engines saturated
  with exactly one 1024-column drain op per PSUM group and nothing else:
  * ALL matmuls run in bf16. Mixing fp32r and bf16 matmuls in one PE
    stream silently corrupts the bf16 weights' last row (hw-verified:
    K=101 bf16 matmuls are exact in a pure-bf16 stream, but lose row 100
    when fp32r self-loading matmuls interleave), and fp32r x bf16 in one
    instruction is rejected outright, so the whole chain is bf16.
  * The stationary free dim is padded to 128 columns (zeros), which
    triggers the compiler's Fast Weight Load (NumWeights==128, non-fp32)
    and hides the per-matmul LDWEIGHTS cost of the explicit-LDW bf16
    path. PSUM partitions 100-127 hold garbage that is never drained.
  * Bias folds into the matmul via a constant ones-row (row 100).
  * Drains for layers 0-8 are single-op ReLU (ACT activation / DVE
    tensor_scalar_max) writing bf16 activations. Layer 9's drain is the
    residual: one DVE scalar_tensor_tensor max(z,0)+trunk -> fp32 trunk
    (exact across all 100 blocks, ping-ponged between two buffers).
  * Pool (idle otherwise - it cannot touch PSUM) casts the fp32 trunk to
    the bf16 copy that feeds the next block's layer 0 and, at the very
    end, the final Linear.
  * A build-time greedy balancer splits drain groups ACT:DVE by modeled
    per-op cost (ACT ~1147ns, DVE ~1220ns per [100,1024] drain).
- Numerics: bf16 weights/activations with an exact fp32 trunk;
  rel err ~1.5e-3 vs the 2e-2 gate.
"""

import sys

sys.path.insert(0, "/opt/trn_rl_repo")

import numpy as np
import ml_dtypes
from contextlib import ExitStack

import concourse.bass as bass
import concourse.bacc as bacc
import concourse.tile as tile
from concourse import mybir
from concourse.bass_utils import run_bass_kernel_spmd

N_BLOCKS = 100
LAYERS_PER_BLOCK = 10
D = 100
D_OUT = 10
BATCH = 65536
N_CORES = 8
B_CORE = BATCH // N_CORES  # 8192 batch columns per core
KAUG = D + 1  # 100 weight rows + 1 bias row
MPAD = 128    # stationary free dim padded for Fast Weight Load

F32 = mybir.dt.float32
BF16 = mybir.dt.bfloat16
BFNP = ml_dtypes.bfloat16

GROUP = 1024            # columns per PSUM tile / drain op
N_GROUPS = B_CORE // GROUP  # 8
MM_N = 512              # moving free dim per matmul
ALU = mybir.AluOpType
ACTF = mybir.ActivationFunctionType


class _EngineBalancer:
    """Greedy static assignment of drain ops to ACT/DVE using modeled
    per-op engine costs (ns)."""

    def __init__(self):
        self.load = {"act": 0.0, "dve": 0.0}

    def pick(self, costs: dict) -> str:
        best = min(costs, key=lambda e: self.load[e] + costs[e])
        self.load[best] += costs[best]
        return best


# modeled per-1024-col op costs (ns), calibrated from baseline traces
_C_DRAIN = {"act": 1147.0, "dve": 1220.0}
_C_STT = {"dve": 1220.0}
_C_FCOPY = {"act": 1147.0, "dve": 1220.0}


def _build(n_blocks: int = N_BLOCKS, b_core: int = B_CORE):
    n_groups = b_core // GROUP
    nc = bacc.Bacc("TRN2", target_bir_lowering=False, debug=False,
                   num_devices=N_CORES)

    xtf = nc.dram_tensor("xtf", [KAUG, b_core], F32, kind="ExternalInput").ap()
    xtb = nc.dram_tensor("xtb", [KAUG, b_core], BF16, kind="ExternalInput").ap()
    obf = nc.dram_tensor("obf", [1, b_core], BF16, kind="ExternalInput").ap()
    of32 = nc.dram_tensor("of32", [1, b_core], F32, kind="ExternalInput").ap()
    wb = nc.dram_tensor("wb", [n_blocks, KAUG, LAYERS_PER_BLOCK, MPAD], BF16,
                        kind="ExternalInput").ap()
    wf = nc.dram_tensor("wf", [KAUG, MPAD], BF16, kind="ExternalInput").ap()
    out = nc.dram_tensor("out", [D_OUT, b_core], F32,
                         kind="ExternalOutput").ap()

    bal = _EngineBalancer()

    with tile.TileContext(nc) as tc, ExitStack() as ctx:
        acts = ctx.enter_context(tc.tile_pool(name="acts", bufs=1))
        wpool = ctx.enter_context(tc.tile_pool(name="w", bufs=2))
        wfpool = ctx.enter_context(tc.tile_pool(name="wf", bufs=1))
        opool = ctx.enter_context(tc.tile_pool(name="o", bufs=1))
        psum = ctx.enter_context(tc.tile_pool(name="ps", bufs=4, space="PSUM"))

        t_a = acts.tile([KAUG, b_core], BF16, tag="ta", name="ta")
        t_b = acts.tile([KAUG, b_core], BF16, tag="tb", name="tb")
        trunk8 = acts.tile([KAUG, b_core], BF16, tag="trunk8", name="trunk8")
        trunk_a = acts.tile([KAUG, b_core], F32, tag="trunka", name="trunka")
        trunk_b = acts.tile([KAUG, b_core], F32, tag="trunkb", name="trunkb")
        out_sb = opool.tile([D_OUT, b_core], F32, tag="outsb", name="outsb")
        wf_sb = wfpool.tile([KAUG, MPAD], BF16, tag="wfsb", name="wfsb")

        # initial loads: fp32 trunk + its bf16 copy (both with ones rows),
        # bf16 ones rows for the ping-pong buffers, fp32 ones for trunk_b.
        nc.gpsimd.dma_start(trunk_a[:, :], xtf[:, :])
        nc.gpsimd.dma_start(trunk8[:, :], xtb[:, :])
        nc.gpsimd.dma_start(t_a[D:KAUG, :], obf[:, :])
        nc.gpsimd.dma_start(t_b[D:KAUG, :], obf[:, :])
        nc.gpsimd.dma_start(trunk_b[D:KAUG, :], of32[:, :])
        nc.gpsimd.dma_start(wf_sb[:, :], wf[:, :])

        def drain_relu(eng, dst_ap, ps_ap):
            if eng == "act":
                nc.scalar.activation(dst_ap, ps_ap, ACTF.Relu)
            else:
                nc.vector.tensor_scalar_max(dst_ap, ps_ap, 0.0)

        tr_old, tr_new = trunk_a, trunk_b
        for bl in range(n_blocks):
            wtb = wpool.tile([KAUG, LAYERS_PER_BLOCK, MPAD], BF16, tag="wtb")
            nc.gpsimd.dma_start(wtb[:, :, :], wb[bl, :, :, :])

            for layer in range(LAYERS_PER_BLOCK):
                last = layer == LAYERS_PER_BLOCK - 1
                if layer == 0:
                    cur = trunk8
                else:
                    cur = t_a if layer % 2 == 1 else t_b
                w_l = wtb[:, layer, :]
                dst = t_b if layer % 2 == 1 else t_a
                for g in range(n_groups):
                    ps = psum.tile([MPAD, GROUP], F32, tag="ps")
                    for h in range(GROUP // MM_N):
                        c0 = g * GROUP + h * MM_N
                        nc.tensor.matmul(
                            ps[:, h * MM_N:(h + 1) * MM_N],
                            w_l,
                            cur[:, c0:c0 + MM_N],
                            start=True, stop=True,
                        )
                    gs = slice(g * GROUP, (g + 1) * GROUP)
                    if last:
                        # residual: trunk' = max(z,0) + trunk, exact fp32
                        bal.pick(_C_STT)
                        nc.vector.scalar_tensor_tensor(
                            tr_new[0:D, gs], ps[0:D, :], 0.0, tr_old[0:D, gs],
                            op0=ALU.max, op1=ALU.add)
                        # bf16 trunk copy for the next block's layer 0 /
                        # the final Linear, on the otherwise-idle Pool
                        nc.gpsimd.tensor_copy(trunk8[0:D, gs],
                                              tr_new[0:D, gs])
                    else:
                        drain_relu(bal.pick(_C_DRAIN), dst[0:D, gs],
                                   ps[0:D, :])
            tr_old, tr_new = tr_new, tr_old

        # Final Linear(100 -> 10), bf16 off the bf16 trunk copy.
        for g in range(n_groups):
            ps = psum.tile([MPAD, GROUP], F32, tag="ps")
            for h in range(GROUP // MM_N):
                c0 = g * GROUP + h * MM_N
                nc.tensor.matmul(ps[:, h * MM_N:(h + 1) * MM_N],
                                 wf_sb[:, :],
                                 trunk8[:, c0:c0 + MM_N],
                                 start=True, stop=True)
            gs = slice(g * GROUP, (g + 1) * GROUP)
            eng = bal.pick(_C_FCOPY)
            if eng == "act":
                nc.scalar.copy(out_sb[:, gs], ps[0:D_OUT, :])
            else:
                nc.vector.tensor_copy(out_sb[:, gs], ps[0:D_OUT, :])
        nc.gpsimd.dma_start(out[:, :], out_sb[:, :])

    nc.compile()
    return nc


def _prep_inputs(x, W, b, Wf, bf):
    """Host-side quantize/reshape; returns per-core input maps."""
    nb, lpb = N_BLOCKS, LAYERS_PER_BLOCK

    # bf16 weights, stationary free dim padded to MPAD for FWL:
    # wb[k, bl, l, m] = W[bl,l,m,k]; k=100 -> b[bl,l,m]; m>=100 -> 0
    wbq = np.zeros((nb, KAUG, lpb, MPAD), BFNP)
    wbq[:, :D, :, :D] = W.transpose(0, 3, 1, 2).astype(BFNP)
    wbq[:, D, :, :D] = b.astype(BFNP)

    wf_ = np.zeros((KAUG, MPAD), BFNP)
    wf_[:D, :D_OUT] = Wf.T.astype(BFNP)
    wf_[D, :D_OUT] = bf.astype(BFNP)

    xtf = np.empty((KAUG, BATCH), np.float32)
    xtf[:D] = x.T
    xtf[D] = 1.0
    xtb = xtf.astype(BFNP)

    obf = np.ones((1, B_CORE), BFNP)
    of32 = np.ones((1, B_CORE), np.float32)

    in_maps = []
    for c in range(N_CORES):
        sl = slice(c * B_CORE, (c + 1) * B_CORE)
        in_maps.append({
            "xtf": np.ascontiguousarray(xtf[:, sl]),
            "xtb": np.ascontiguousarray(xtb[:, sl]),
            "obf": obf,
            "of32": of32,
            "wb": wbq,
            "wf": wf_,
        })
    return in_maps


_CACHED_NC = None


def kernel(x, W, b, Wf, bf, _trace=False, _trace_kwargs=None):
    global _CACHED_NC
    in_maps = _prep_inputs(np.asarray(x, np.float32), np.asarray(W, np.float32),
                           np.asarray(b, np.float32), np.asarray(Wf, np.float32),
                           np.asarray(bf, np.float32))
    if _CACHED_NC is None:
        _CACHED_NC = _build()
    nc = _CACHED_NC
    kw = dict(_trace_kwargs or {})
    res = run_bass_kernel_spmd(nc, in_maps, core_ids=list(range(N_CORES)),
                               trace=_trace, **kw)
    outs = [res.results[c]["out"] for c in range(N_CORES)]  # [10, 8192] each
    full = np.concatenate(outs, axis=1).T  # [65536, 10]
    if _trace:
        kernel.last_results = res
    return np.ascontiguousarray(full)


# revision 13
# speedup vs baseline: 1.6287x; 1.6287x over previous
"""BigResNet Trainium2 kernel.

Computation (see reference): x:[65536,100]; 100 blocks of
(10x Linear(100,100)+ReLU) with a residual add per block; final Linear(100,10).

Strategy:
- Data-parallel over the batch: 8 cores x 8192 rows each.
- Activations live in SBUF transposed: [D=100 (+1 ones row), batch]. The
  contraction dim D sits on SBUF partitions for both matmul operands, so no
  transposes are needed anywhere in the layer chain.
- Bias is folded into the matmul via a constant ones-row at partition 100 and
  an extra weight row (K=101).
- Weights are host-side rearranged to [101, block, layer*100] so each block's
  weights DMA as 101 partitions x 4000B contiguous lines.
- Matmul dtype float32r (fp32 truncated to FP22 inside the PE): full PE rate,
  ~2^-12 relative precision, fp32 accumulate in PSUM.
- ReLU drains PSUM->SBUF, split between ScalarE (activation) and VectorE
  (tensor_scalar_max) by a build-time greedy balance on measured per-op
  costs (ACT ~1119ns, DVE ~1214ns per [100,1024] drain; DVE also owns the
  8 per-block residual STTs). The block-residual is fused into the last
  layer's drain as one VectorE scalar_tensor_tensor: out = max(z,0) + x.
"""

import sys

sys.path.insert(0, "/opt/trn_rl_repo")

import numpy as np
from contextlib import ExitStack

import concourse.bass as bass
import concourse.bacc as bacc
import concourse.tile as tile
from concourse import mybir
from concourse.bass_utils import run_bass_kernel_spmd
from concourse import bass_utils as _bu


def _enable_ldw_opt():
    """walrus ships with --enable-ldw-opt=false; our inner loop issues 16
    matmuls per weight load, so redundant LDWEIGHTS cost ~80ns/matmul.
    Rewrite the flag on the walrus command line."""
    if getattr(_bu, "_ldw_opt_patched", False):
        return
    _orig = _bu.run_command

    def run_command(cmd, *a, **k):
        cmd = ["--enable-ldw-opt=true" if c == "--enable-ldw-opt=false" else c
               for c in cmd]
        return _orig(cmd, *a, **k)

    _bu.run_command = run_command
    _bu._ldw_opt_patched = True


_enable_ldw_opt()

N_BLOCKS = 100
LAYERS_PER_BLOCK = 10
D = 100
D_OUT = 10
BATCH = 65536
N_CORES = 8
B_CORE = BATCH // N_CORES  # 8192 batch columns per core
KAUG = D + 1  # 100 weight rows + 1 bias row

F32 = mybir.dt.float32
F32R = mybir.dt.float32r

# Column-group size for the PSUM->SBUF drain ops (ReLU / residual).
GROUP = 1024
N_GROUPS = B_CORE // GROUP  # 8
MM_N = 512  # max moving-operand free dim for fp32
MM_PER_GROUP = GROUP // MM_N  # 2

# Greedy ACT/DVE balance on measured per-op costs. DVE additionally owns
# the 8 per-block residual STTs (ACT cannot do tensor+tensor).
_C_ACT = 1119.0
_C_DVE = 1214.0


class _Balancer:
    def __init__(self):
        self.act = 0.0
        self.dve = 0.0

    def pick_drain(self) -> bool:
        """True -> ACT."""
        if self.act + _C_ACT <= self.dve + _C_DVE:
            self.act += _C_ACT
            return True
        self.dve += _C_DVE
        return False

    def add_stt(self):
        self.dve += _C_DVE


def _build(n_blocks: int = N_BLOCKS, b_core: int = B_CORE):
    n_groups = b_core // GROUP
    nc = bacc.Bacc("TRN2", target_bir_lowering=False, debug=False,
                   num_devices=N_CORES)

    xt = nc.dram_tensor("xt", [KAUG, b_core], F32R, kind="ExternalInput").ap()
    wa = nc.dram_tensor("wa", [KAUG, n_blocks, LAYERS_PER_BLOCK * D], F32R,
                        kind="ExternalInput").ap()
    wf = nc.dram_tensor("wf", [KAUG, D_OUT], F32R, kind="ExternalInput").ap()
    out = nc.dram_tensor("out", [D_OUT, b_core], F32,
                         kind="ExternalOutput").ap()

    with tile.TileContext(nc) as tc, ExitStack() as ctx:
        acts = ctx.enter_context(tc.tile_pool(name="acts", bufs=1))
        wpool = ctx.enter_context(tc.tile_pool(name="w", bufs=2))
        wfpool = ctx.enter_context(tc.tile_pool(name="wf", bufs=1))
        opool = ctx.enter_context(tc.tile_pool(name="o", bufs=1))
        psum = ctx.enter_context(tc.tile_pool(name="ps", bufs=4, space="PSUM"))

        # Three resident activation buffers, rotated across blocks.
        bufs = [acts.tile([KAUG, b_core], F32R, tag=f"act{i}", name=f"act{i}")
                for i in range(3)]
        # x lands in bufs[0]; host ships the ones-row as row 100 of xt.
        nc.gpsimd.dma_start(bufs[0][:, :], xt[:, :])
        # The temp buffers need their ones-row too (ReLU only writes rows
        # 0:100); copy it from xt's ones-row.
        nc.gpsimd.dma_start(bufs[1][D:KAUG, :], xt[D:KAUG, :])
        nc.gpsimd.dma_start(bufs[2][D:KAUG, :], xt[D:KAUG, :])

        wf_sb = wfpool.tile([KAUG, D_OUT], F32R)
        nc.gpsimd.dma_start(wf_sb[:, :], wf[:, :])

        bal = _Balancer()
        x_buf, t1, t2 = bufs[0], bufs[1], bufs[2]
        for bl in range(n_blocks):
            wt = wpool.tile([KAUG, LAYERS_PER_BLOCK * D], F32R, tag="wt")
            nc.gpsimd.dma_start(wt[:, :], wa[:, bl, :])

            cur = x_buf
            for layer in range(LAYERS_PER_BLOCK):
                gl = bl * LAYERS_PER_BLOCK + layer
                w_l = wt[:, layer * D:(layer + 1) * D]
                last = layer == LAYERS_PER_BLOCK - 1
                dst = t2 if last else (t1 if layer % 2 == 0 else t2)
                for g in range(n_groups):
                    ps = psum.tile([D, GROUP], F32, tag="ps")
                    for h in range(MM_PER_GROUP):
                        c0 = g * GROUP + h * MM_N
                        nc.tensor.matmul(
                            ps[:, h * MM_N:(h + 1) * MM_N],
                            w_l,
                            cur[:, c0:c0 + MM_N],
                            start=True, stop=True,
                        )
                    gs = slice(g * GROUP, (g + 1) * GROUP)
                    if last:
                        # x_new = relu(z) + x, one fused DVE op per group
                        bal.add_stt()
                        nc.vector.scalar_tensor_tensor(
                            dst[0:D, gs], ps[:, :], 0.0, x_buf[0:D, gs],
                            op0=mybir.AluOpType.max,
                            op1=mybir.AluOpType.add)
                    elif bal.pick_drain():
                        nc.scalar.activation(
                            dst[0:D, gs], ps[:, :],
                            mybir.ActivationFunctionType.Relu)
                    else:
                        nc.vector.tensor_scalar_max(dst[0:D, gs], ps[:, :], 0.0)
                cur = dst
            # rotate: new x is t2 (holds x+y); old x becomes scratch
            x_buf, t1, t2 = t2, x_buf, t1

        # Final Linear(100 -> 10): psum [10, 1024] tiles, copy to SBUF, DMA out.
        out_sb = opool.tile([D_OUT, b_core], F32)
        for g in range(n_groups):
            ps = psum.tile([D, GROUP], F32, tag="ps")
            for h in range(MM_PER_GROUP):
                c0 = g * GROUP + h * MM_N
                nc.tensor.matmul(ps[0:D_OUT, h * MM_N:(h + 1) * MM_N],
                                 wf_sb[:, :],
                                 x_buf[:, c0:c0 + MM_N],
                                 start=True, stop=True)
            gs = slice(g * GROUP, (g + 1) * GROUP)
            if g % 2 == 0:
                nc.scalar.copy(out_sb[:, gs], ps[0:D_OUT, :])
            else:
                nc.vector.tensor_copy(out_sb[:, gs], ps[0:D_OUT, :])
        nc.gpsimd.dma_start(out[:, :], out_sb[:, :])

    nc.compile()
    return nc


def _prep_inputs(x, W, b, Wf, bf):
    """Host-side reshape/augment; returns per-core input maps."""
    # wa[i, bl, l*100+o]: i<100 -> W[bl,l,o,i]; i==100 -> b[bl,l,o]
    wa = np.empty((KAUG, N_BLOCKS, LAYERS_PER_BLOCK * D), np.float32)
    wt = np.ascontiguousarray(W.transpose(3, 0, 1, 2))  # [i, bl, l, o]
    wa[:D] = wt.reshape(D, N_BLOCKS, LAYERS_PER_BLOCK * D)
    wa[D] = b.reshape(N_BLOCKS, LAYERS_PER_BLOCK * D)

    wfa = np.empty((KAUG, D_OUT), np.float32)
    wfa[:D] = Wf.T
    wfa[D] = bf

    xt = np.empty((KAUG, BATCH), np.float32)
    xt[:D] = x.T
    xt[D] = 1.0

    in_maps = []
    for c in range(N_CORES):
        sl = slice(c * B_CORE, (c + 1) * B_CORE)
        in_maps.append({
            "xt": np.ascontiguousarray(xt[:, sl]),
            "wa": wa,
            "wf": wfa,
        })
    return in_maps


_CACHED_NC = None


def kernel(x, W, b, Wf, bf, _trace=False, _trace_kwargs=None):
    global _CACHED_NC
    x = np.asarray(x, np.float32)
    in_maps = _prep_inputs(np.asarray(x, np.float32), np.asarray(W, np.float32),
                           np.asarray(b, np.float32), np.asarray(Wf, np.float32),
                           np.asarray(bf, np.float32))
    if _CACHED_NC is None:
        _CACHED_NC = _build()
    nc = _CACHED_NC
    kw = dict(_trace_kwargs or {})
    res = run_bass_kernel_spmd(nc, in_maps, core_ids=list(range(N_CORES)),
                               trace=_trace, **kw)
    outs = [res.results[c]["out"] for c in range(N_CORES)]  # [10, 8192] each
    full = np.concatenate(outs, axis=1).T  # [65536, 10]
    if _trace:
        kernel.last_results = res
    return np.ascontiguousarray(full)



# revision 16
# speedup vs baseline: 1.7506x; 1.0748x over previous
"""BigResNet Trainium2 kernel.

Computation (see reference): x:[65536,100]; 100 blocks of
(10x Linear(100,100)+ReLU) with a residual add per block; final Linear(100,10).

Strategy:
- Data-parallel over the batch: 8 cores x 8192 rows each.
- Activations live in SBUF transposed: [D=100 (+1 ones row), batch]. The
  contraction dim D sits on SBUF partitions for both matmul operands, so no
  transposes are needed anywhere in the layer chain.
- Bias is folded into the matmul via a constant ones-row at partition 100 and
  an extra weight row (K=101).
- Weights are host-side rearranged to [101, block, layer*100] so each block's
  weights DMA as 101 partitions x 4000B contiguous lines.
- Matmul dtype float32r (fp32 truncated to FP22 inside the PE): full PE rate,
  ~2^-12 relative precision, fp32 accumulate in PSUM.
- ReLU drains PSUM->SBUF, split between ScalarE (activation) and VectorE
  (tensor_scalar_max) by a build-time greedy balance on measured per-op
  costs (ACT ~1119ns, DVE ~1214ns per [100,1024] drain).
- The block residual never touches ACT/DVE and never gates the block
  boundary: layer 9 drains relu(z9) into a y9 buffer like any other
  layer, the NEXT block's layer-0 matmul consumes (trunk, y9) as two
  accumulating PSUM passes (exact add on the PE), and the fp32 trunk
  itself is materialized lazily by GpSimd SBUF adds that have a whole
  block of slack. The final Linear uses the same two-pass trick.
"""

import sys

sys.path.insert(0, "/opt/trn_rl_repo")

import numpy as np
from contextlib import ExitStack

import concourse.bass as bass
import concourse.bacc as bacc
import concourse.tile as tile
from concourse import mybir
from concourse.bass_utils import run_bass_kernel_spmd
from concourse import bass_utils as _bu


def _enable_ldw_opt():
    """walrus ships with --enable-ldw-opt=false; our inner loop issues 16
    matmuls per weight load, so redundant LDWEIGHTS cost ~80ns/matmul.
    Rewrite the flag on the walrus command line."""
    if getattr(_bu, "_ldw_opt_patched", False):
        return
    _orig = _bu.run_command

    def run_command(cmd, *a, **k):
        cmd = ["--enable-ldw-opt=true" if c == "--enable-ldw-opt=false" else c
               for c in cmd]
        return _orig(cmd, *a, **k)

    _bu.run_command = run_command
    _bu._ldw_opt_patched = True


_enable_ldw_opt()

N_BLOCKS = 100
LAYERS_PER_BLOCK = 10
D = 100
D_OUT = 10
BATCH = 65536
N_CORES = 8
B_CORE = BATCH // N_CORES  # 8192 batch columns per core
KAUG = D + 1  # 100 weight rows + 1 bias row

F32 = mybir.dt.float32
F32R = mybir.dt.float32r

# Column-group size for the PSUM->SBUF drain ops (ReLU / residual).
GROUP = 1024
N_GROUPS = B_CORE // GROUP  # 8
MM_N = 512  # max moving-operand free dim for fp32
MM_PER_GROUP = GROUP // MM_N  # 2

# Greedy ACT/DVE split of the [100,1024] ReLU drains on measured per-op
# costs; every PSUM drain in the kernel is the same op class.
_C_ACT = 1119.0
_C_DVE = 1214.0


class _Balancer:
    def __init__(self):
        self.act = 0.0
        self.dve = 0.0

    def pick_act(self) -> bool:
        if self.act + _C_ACT <= self.dve + _C_DVE:
            self.act += _C_ACT
            return True
        self.dve += _C_DVE
        return False


def _build(n_blocks: int = N_BLOCKS, b_core: int = B_CORE):
    n_groups = b_core // GROUP
    nc = bacc.Bacc("TRN2", target_bir_lowering=False, debug=False,
                   num_devices=N_CORES)

    xt = nc.dram_tensor("xt", [KAUG, b_core], F32R, kind="ExternalInput").ap()
    wa = nc.dram_tensor("wa", [n_blocks, KAUG, LAYERS_PER_BLOCK * D], F32R,
                        kind="ExternalInput").ap()
    wf = nc.dram_tensor("wf", [KAUG, D_OUT], F32R, kind="ExternalInput").ap()
    zrow = nc.dram_tensor("zrow", [1, b_core], F32R, kind="ExternalInput").ap()
    out = nc.dram_tensor("out", [D_OUT, b_core], F32,
                         kind="ExternalOutput").ap()

    with tile.TileContext(nc) as tc, ExitStack() as ctx:
        acts = ctx.enter_context(tc.tile_pool(name="acts", bufs=1))
        wpool = ctx.enter_context(tc.tile_pool(name="w", bufs=2))
        wfpool = ctx.enter_context(tc.tile_pool(name="wf", bufs=1))
        opool = ctx.enter_context(tc.tile_pool(name="o", bufs=1))
        psum = ctx.enter_context(tc.tile_pool(name="ps", bufs=4, space="PSUM"))

        # Residual trunk ping-pong (x_a/x_b), the y9 buffer (last-layer relu
        # output, consumed by the next block's two-pass layer 0 and the lazy
        # GpSimd trunk add), and the in-block ping-pong temps t1/t2.
        x_a = acts.tile([KAUG, b_core], F32R, tag="xa", name="xa")
        x_b = acts.tile([KAUG, b_core], F32R, tag="xb", name="xb")
        y9 = acts.tile([KAUG, b_core], F32R, tag="y9", name="y9")
        t1 = acts.tile([KAUG, b_core], F32R, tag="t1", name="t1")
        t2 = acts.tile([KAUG, b_core], F32R, tag="t2", name="t2")
        # x_a holds x with its ones-row (host ships row 100 of xt = 1).
        nc.gpsimd.dma_start(x_a[:, :], xt[:, :])
        # t1/t2/x_b need the ones-row too (drains only write rows 0:100).
        nc.gpsimd.dma_start(t1[D:KAUG, :], xt[D:KAUG, :])
        nc.gpsimd.dma_start(t2[D:KAUG, :], xt[D:KAUG, :])
        nc.gpsimd.dma_start(x_b[D:KAUG, :], xt[D:KAUG, :])
        # y9 row 100 must be ZERO: in the two-pass layer 0 the bias enters
        # via the trunk pass; a second ones-row would double it.
        nc.gpsimd.dma_start(y9[D:KAUG, :], zrow[:, :])

        wf_sb = wfpool.tile([KAUG, D_OUT], F32R)
        nc.gpsimd.dma_start(wf_sb[:, :], wf[:, :])

        bal = _Balancer()
        x_old, x_new = x_a, x_b
        for bl in range(n_blocks):
            wt = wpool.tile([KAUG, LAYERS_PER_BLOCK * D], F32R, tag="wt")
            nc.gpsimd.dma_start(wt[:, :], wa[bl, :, :])

            if bl > 0:
                # lazy fp32 trunk materialization on the otherwise-idle
                # GpSimd: x_bl = x_{bl-1} + y9_{bl-1}. Runs concurrently
                # with this block's layers (everything it reads is stable
                # until layer 9 overwrites y9), never gates the boundary.
                for g in range(n_groups):
                    gs = slice(g * GROUP, (g + 1) * GROUP)
                    nc.gpsimd.tensor_add(
                        x_new[0:D, gs], x_old[0:D, gs], y9[0:D, gs])

            cur = None
            for layer in range(LAYERS_PER_BLOCK):
                w_l = wt[:, layer * D:(layer + 1) * D]
                last = layer == LAYERS_PER_BLOCK - 1
                dst = y9 if last else (t1 if layer % 2 == 0 else t2)
                for g in range(n_groups):
                    ps = psum.tile([D, GROUP], F32, tag="ps")
                    for h in range(MM_PER_GROUP):
                        c0 = g * GROUP + h * MM_N
                        sl = slice(c0, c0 + MM_N)
                        po = ps[:, h * MM_N:(h + 1) * MM_N]
                        if layer > 0:
                            nc.tensor.matmul(po, w_l, cur[:, sl],
                                             start=True, stop=True)
                        elif bl == 0:
                            nc.tensor.matmul(po, w_l, x_old[:, sl],
                                             start=True, stop=True)
                        else:
                            # two-pass layer 0: psum = W0 x + W0 y9_prev;
                            # the residual add happens exactly, in PSUM.
                            nc.tensor.matmul(po, w_l, x_old[:, sl],
                                             start=True, stop=False)
                            nc.tensor.matmul(po, w_l, y9[:, sl],
                                             start=False, stop=True)
                    gs = slice(g * GROUP, (g + 1) * GROUP)
                    if bal.pick_act():
                        nc.scalar.activation(
                            dst[0:D, gs], ps[:, :],
                            mybir.ActivationFunctionType.Relu)
                    else:
                        nc.vector.tensor_scalar_max(dst[0:D, gs], ps[:, :], 0.0)
                cur = dst
            if bl > 0:
                x_old, x_new = x_new, x_old

        # Final Linear(100 -> 10): out = Wf x_99 + Wf y9 (two-pass, so the
        # last trunk materialization is never needed); [10, 1024] drains.
        out_sb = opool.tile([D_OUT, b_core], F32)
        for g in range(n_groups):
            ps = psum.tile([D, GROUP], F32, tag="ps")
            for h in range(MM_PER_GROUP):
                c0 = g * GROUP + h * MM_N
                sl = slice(c0, c0 + MM_N)
                po = ps[0:D_OUT, h * MM_N:(h + 1) * MM_N]
                nc.tensor.matmul(po, wf_sb[:, :], x_old[:, sl],
                                 start=True, stop=False)
                nc.tensor.matmul(po, wf_sb[:, :], y9[:, sl],
                                 start=False, stop=True)
            gs = slice(g * GROUP, (g + 1) * GROUP)
            if g % 2 == 0:
                nc.scalar.copy(out_sb[:, gs], ps[0:D_OUT, :])
            else:
                nc.vector.tensor_copy(out_sb[:, gs], ps[0:D_OUT, :])
        nc.gpsimd.dma_start(out[:, :], out_sb[:, :])

    nc.compile()
    return nc


def _prep_inputs(x, W, b, Wf, bf):
    """Host-side reshape/augment; returns per-core input maps."""
    # wa[i, bl, l*100+o]: i<100 -> W[bl,l,o,i]; i==100 -> b[bl,l,o]
    # Block-major [nb, KAUG, ...]: per-block DMA slices are contiguous.
    # (A [KAUG, nb, ...] layout with huge partition strides triggers a DMA/
    # upload row-corruption bug - SBUF rows land permuted - hw-verified.)
    wa = np.empty((N_BLOCKS, KAUG, LAYERS_PER_BLOCK * D), np.float32)
    wt = np.ascontiguousarray(W.transpose(0, 3, 1, 2))  # [bl, i, l, o]
    wa[:, :D] = wt.reshape(N_BLOCKS, D, LAYERS_PER_BLOCK * D)
    wa[:, D] = b.reshape(N_BLOCKS, LAYERS_PER_BLOCK * D)

    wfa = np.empty((KAUG, D_OUT), np.float32)
    wfa[:D] = Wf.T
    wfa[D] = bf

    xt = np.empty((KAUG, BATCH), np.float32)
    xt[:D] = x.T
    xt[D] = 1.0

    zrow = np.zeros((1, B_CORE), np.float32)
    in_maps = []
    for c in range(N_CORES):
        sl = slice(c * B_CORE, (c + 1) * B_CORE)
        in_maps.append({
            "xt": np.ascontiguousarray(xt[:, sl]),
            "wa": wa,
            "wf": wfa,
            "zrow": zrow,
        })
    return in_maps


_CACHED_NC = None


def kernel(x, W, b, Wf, bf, _trace=False, _trace_kwargs=None):
    global _CACHED_NC
    x = np.asarray(x, np.float32)
    in_maps = _prep_inputs(np.asarray(x, np.float32), np.asarray(W, np.float32),
                           np.asarray(b, np.float32), np.asarray(Wf, np.float32),
                           np.asarray(bf, np.float32))
    if _CACHED_NC is None:
        _CACHED_NC = _build()
    nc = _CACHED_NC
    kw = dict(_trace_kwargs or {})
    res = run_bass_kernel_spmd(nc, in_maps, core_ids=list(range(N_CORES)),
                               trace=_trace, **kw)
    outs = [res.results[c]["out"] for c in range(N_CORES)]  # [10, 8192] each
    full = np.concatenate(outs, axis=1).T  # [65536, 10]
    if _trace:
        kernel.last_results = res
    return np.ascontiguousarray(full)

